# revision 26
# baseline (speedup 1.0000x reference)
"""Causal attention kernel for TRN2, sharded over batch*heads on 8 NeuronCores.

Problem: B=2, H=16, S=2048, D=64, f32 causal scaled-dot-product attention.

Strategy (per core: 4 heads = 2 head-pairs):
  - Host pre-transposes Q, K to [D, S] (d on partitions), packs two heads
    per 128-partition tile (head A on partitions 0:64, head B on 64:128),
    casts to bf16 (PE runs bf16 at 1 cyc/row vs 4 for f32).
  - QK^T for the two heads runs as two concurrent row-tiled matmuls
    (tile_position auto-derived from base_partition 0 / 64).
  - Host appends a ones-column to V so the softmax denominator falls out of
    the same PE matmul that computes exp(S)@V (M = 65 stationary columns).
  - Work unit: (pair, q-quarter qq of 512, k-tile kt<=4qq+3) strip of
    scoresT [128 k, 2 heads, W<=512 q] in PSUM; one exp covers both
    heads via a [128, 2, W] access pattern straight out of PSUM (scale=1/8
    folded in); no max-subtraction (scores ~ N(0,1), exp cannot overflow);
    diagonal 128x128 blocks masked by one bf16 triu multiply on VectorE for
    both heads.
  - exp is split between ScalarE (true exp ACTIVATE) and VectorE (Schraudolph
    bf16-bit-pattern exp2 via one tensor_scalar mult+add, ~3% elementwise
    error that softmax renormalization mostly cancels): ScalarE alone is the
    bottleneck at ~75us busy while VectorE idles, so every DVE_STRIDE-th
    off-diagonal strip moves to VectorE.
  - PSUM (8 banks): scores triple-buffered (3 x 2 banks) so QK always runs
    two groups ahead of exp; one single-buffered [65, 2, 512] out accumulator
    (2 banks).
  - Startup: PE comes out of reset HAM-throttled (K=4/8, half rate) and only
    unthrottles after sustained dense activity; a zero warmup matmul block
    starts PE activity at ~7us (vs ~10) to pull the unthrottle point in.
    First-strip DMAs (k-tile 0, q quarter 0, mask) are issued first and small.
  - Tail: the final quarter drains cols 0:128 right after their last
    contributing k-tile instead of waiting for the full quarter.
  - Device ships unnormalized [65, S] per head (rows 0-63 numerator^T,
    row 64 denominator); host divides and transposes back.
"""

import numpy as np
import ml_dtypes

B, H, S, D = 2, 16, 2048, 64
NCORES = 8
HPC = (B * H) // NCORES  # heads per core = 4
NPAIR = HPC // 2  # head pairs per core = 2
NKT = S // 128  # 16 k-tiles per head
QQ = 512  # q quarter width (one PSUM bank per head)
DVE_NUM, DVE_DEN = 1, 2  # offload DVE_NUM of every DVE_DEN off-diag strips to VectorE
GPSIMD_MASK = False  # run the diagonal mask multiplies on the idle GPSIMD
NWARM = 0  # zero matmuls at program start to release the PE HAM throttle
NQQ = S // QQ
BF16 = ml_dtypes.bfloat16

_prog = None


def _build_program():
    import concourse.tile as tile
    from concourse import bacc, mybir

    nc = bacc.Bacc(
        "TRN2",
        target_bir_lowering=False,
        debug=False,
        enable_asserts=False,
        num_devices=NCORES,
    )
    # paired layouts: [pair, 128, S] with head 2p on partitions 0:64, head
    # 2p+1 on partitions 64:128
    qT = nc.dram_tensor("qT", [NPAIR, 128, S], mybir.dt.bfloat16, kind="ExternalInput").ap()
    kT = nc.dram_tensor("kT", [NPAIR, 128, S], mybir.dt.bfloat16, kind="ExternalInput").ap()
    vp = nc.dram_tensor("vp", [HPC, 128, NKT, D + 1], mybir.dt.bfloat16, kind="ExternalInput").ap()
    mk = nc.dram_tensor("mk", [128, 128], mybir.dt.bfloat16, kind="ExternalInput").ap()
    o = nc.dram_tensor("o", [HPC, D + 1, S], mybir.dt.float32, kind="ExternalOutput").ap()

    with tile.TileContext(nc) as tc:
        with (
            tc.tile_pool(name="inputs", bufs=1) as inputs,
            tc.tile_pool(name="expp", bufs=12) as expp,
            tc.tile_pool(name="scp", bufs=3, space="PSUM") as scp,
            tc.tile_pool(name="outp", bufs=1, space="PSUM") as outp,
            tc.tile_pool(name="outsb", bufs=6) as outsb,
        ):
            mkt = inputs.tile([128, 128], mybir.dt.bfloat16, tag="mask")
            qts, kts_, vts = [], [], []
            for p in range(NPAIR):
                qt = inputs.tile([128, S], mybir.dt.bfloat16, tag=f"q{p}")
                kt = inputs.tile([128, S], mybir.dt.bfloat16, tag=f"k{p}")
                va = inputs.tile([128, NKT, D + 1], mybir.dt.bfloat16, tag=f"va{p}")
                vb = inputs.tile([128, NKT, D + 1], mybir.dt.bfloat16, tag=f"vb{p}")
                qts.append(qt)
                kts_.append(kt)
                vts.append((va, vb))
            # The first strip (pair 0, quarter 0, k-tile 0) needs k0's first
            # k-tile, q0's first quarter, the mask and V k-tile 0. Issue those
            # first, split across the two HWDGE queues (sync + scalar);
            # everything else follows behind in the same FIFOs so it cannot
            # steal bandwidth from the critical-path transfers.
            nc.sync.dma_start(kts_[0][:, 0:128], kT[0][:, 0:128])
            nc.scalar.dma_start(qts[0][:, 0:256], qT[0][:, 0:256])
            nc.sync.dma_start(mkt[:], mk)
            nc.scalar.dma_start(qts[0][:, 256:QQ], qT[0][:, 256:QQ])
            nc.scalar.dma_start(vts[0][1][:, 0:4], vp[1][:, 0:4])
            nc.sync.dma_start(vts[0][0][:, 0:4], vp[0][:, 0:4])
            nc.sync.dma_start(kts_[0][:, 128:QQ], kT[0][:, 128:QQ])
            nc.sync.dma_start(kts_[0][:, QQ : 2 * QQ], kT[0][:, QQ : 2 * QQ])
            nc.sync.dma_start(qts[0][:, QQ : 2 * QQ], qT[0][:, QQ : 2 * QQ])
            nc.sync.dma_start(vts[0][0][:, 4:8], vp[0][:, 4:8])
            nc.sync.dma_start(vts[0][1][:, 4:8], vp[1][:, 4:8])
            nc.sync.dma_start(kts_[1][:, 0:QQ], kT[1][:, 0:QQ])
            nc.sync.dma_start(qts[1][:, 0:QQ], qT[1][:, 0:QQ])
            nc.sync.dma_start(vts[1][0][:, 0:4], vp[2][:, 0:4])
            nc.sync.dma_start(vts[1][1][:, 0:4], vp[3][:, 0:4])
            nc.sync.dma_start(kts_[0][:, 2 * QQ : S], kT[0][:, 2 * QQ : S])
            nc.sync.dma_start(qts[0][:, 2 * QQ : S], qT[0][:, 2 * QQ : S])
            nc.sync.dma_start(vts[0][0][:, 8:NKT], vp[0][:, 8:NKT])
            nc.sync.dma_start(vts[0][1][:, 8:NKT], vp[1][:, 8:NKT])
            nc.sync.dma_start(kts_[1][:, QQ:S], kT[1][:, QQ:S])
            nc.sync.dma_start(qts[1][:, QQ:S], qT[1][:, QQ:S])
            nc.sync.dma_start(vts[1][0][:, 4:NKT], vp[2][:, 4:NKT])
            nc.sync.dma_start(vts[1][1][:, 4:NKT], vp[3][:, 4:NKT])

            # PE warmup: the HAM throttle boots at K=4/8 (half rate) and only
            # releases after sustained activity. Burn NWARM zero matmuls into
            # the (not yet used) out accumulator buffer so PE activity starts
            # as soon as the engines come up, well before the first real QK's
            # input DMA lands.
            if NWARM:
                # W=128 zero matmuls into the sc pool's first slot: they run
                # 7.0-8.3us while the first input DMA is still in flight, so
                # the HAM activity clock starts ~1.3us earlier at zero cost
                # (slot 0's next real user only runs at ~12us).
                warm_sb = inputs.tile([128, 128], mybir.dt.bfloat16, tag="warm")
                nc.gpsimd.memset(warm_sb[:], 0.0)
                warm_ps = scp.tile(
                    [128, 2, QQ], mybir.dt.float32, tag="sc", name="warm_ps"
                )
                for _ in range(NWARM):
                    nc.tensor.matmul(
                        warm_ps[:, 0, 0:128],
                        warm_sb[:],
                        warm_sb[:],
                        start=True,
                        stop=True,
                        skip_group_check=True,
                    )

            osbs = {}
            for p in range(NPAIR):
                for jj in range(2):
                    osbs[(p, jj)] = outsb.tile(
                        [D + 1, S], mybir.dt.float32, tag="osb", name=f"osb{p}_{jj}"
                    )
            # interleave the two pairs' quarters so both engines always have
            # independent work to fill dependency gaps
            order = [(0, 0), (0, 1), (1, 0), (0, 2), (1, 1), (0, 3), (1, 2), (1, 3)]
            # flatten (pair, quarter) into a list of strip groups; each group
            # is one score tile + one exp call covering one or two k-tiles
            # (the W=384 and W=128 diagonal strips share a tile). Entries are
            # (kti, soff, qstart, W); av_start flags are derived below from
            # 128-col coverage so the first AV touching each column range
            # resets PSUM.
            all_groups = []
            for p, qq in order:
                q0 = QQ * qq

                def ent(kti, soff, qstart=None, W=None, q0=q0):
                    qs = max(q0, 128 * kti) if qstart is None else qstart
                    w = q0 + QQ - qs if W is None else W
                    return (kti, soff, qs, w)

                if (p, qq) == (0, 0):
                    # split the first diagonal strip in half so the very first
                    # QK only needs q[0:256] + k-tile 0 from DRAM: the first
                    # exp fires ~1us earlier
                    groups = [
                        [ent(0, 0, 0, 256)],
                        [ent(0, 0, 256, 256)],
                        [ent(2, 0)],
                        [ent(1, 0), ent(3, 384)],
                    ]
                elif (p, qq) == order[-1]:
                    # last unit: order the diagonal strips so output columns
                    # complete left-to-right and can drain in three stages
                    groups = [[ent(kti, 0)] for kti in range(4 * qq)]
                    groups.append([ent(4 * qq, 0)])
                    groups.append([ent(4 * qq + 1, 0), ent(4 * qq + 3, 384)])
                    groups.append([ent(4 * qq + 2, 0)])
                else:
                    groups = [[ent(kti, 0)] for kti in range(4 * qq)]
                    groups.append([ent(4 * qq, 0)])  # W=512 diagonal
                    groups.append([ent(4 * qq + 2, 0)])  # W=256 diagonal
                    groups.append([ent(4 * qq + 1, 0), ent(4 * qq + 3, 384)])
                for gi, group in enumerate(groups):
                    # start=True lazily marks the whole 2KB PSUM bank as
                    # pending-zero, so only the quarter's very first AV needs
                    # it; later writes to untouched bytes still zero-fill.
                    entries = [
                        (kti, soff, qs, w, gi == 0 and ei == 0)
                        for ei, (kti, soff, qs, w) in enumerate(group)
                    ]
                    is_diag = all(128 * kti >= q0 for kti, _, _, _, _ in entries)
                    all_groups.append((p, qq, gi, len(groups), entries, is_diag))

            # Fast-exp2 constants (Schraudolph): int16(x*A + B) reinterpreted
            # as bf16 is 2^(x*log2e/8) ~ exp(x/8)*(1 +- 3%); the softmax
            # renormalization cancels most of the approximation error.
            EXP2_A = 128.0 / float(np.log(2.0)) / 8.0
            EXP2_B = 16256.0 - 366393.0 / 65536.0

            def emit_qk(p, entries):
                qt, kt = qts[p], kts_[p]
                sc = scp.tile(
                    [128, 2, QQ], mybir.dt.float32, tag="sc", name="sc_tile"
                )
                for kti, soff, qstart, W, _ in entries:
                    for j in range(2):
                        pb = 64 * j
                        nc.tensor.matmul(
                            sc[:, j, soff : soff + W],
                            kt[pb : pb + 64, 128 * kti : 128 * kti + 128],
                            qt[pb : pb + 64, qstart : qstart + W],
                            start=True,
                            stop=True,
                        )
                return sc

            def drain(p, qq, j, c0, c1, out_t, eng, dmaq):
                """Copy out cols [c0:c1) of head j to SBUF and DMA them out."""
                q0 = QQ * qq
                eng.copy(
                    osbs[(p, j)][:, q0 + c0 : q0 + c1], out_t[:, j, c0:c1]
                )
                dmaq.dma_start(
                    o[2 * p + j][:, q0 + c0 : q0 + c1],
                    osbs[(p, j)][:, q0 + c0 : q0 + c1],
                )

            nd_counter = 0
            out_ts = {}
            mask_eng = nc.gpsimd if GPSIMD_MASK else nc.vector
            # Software-pipeline: QK runs two groups ahead of exp and AV runs
            # one group behind it. The scheduler is a priority heap keyed by
            # emission order, so emitting QK(i+2) before AV(i-1) lets PE
            # start the next unit's scores while the previous unit's AV
            # backlog drains — otherwise ScalarE starves ~2.5us at every
            # (pair, quarter) boundary. Lagging AV by one group means the AV
            # almost never waits on its exp, so the PE pipeline doesn't drain
            # (each exposed drain costs the 173ns SBUF access latency).
            LOOKAHEAD = 2
            AV_LAG = 1
            hoisted = {}
            pending = {}

            def emit_av(i):
                p, qq, gi, ng, entries, ex = pending.pop(i)
                q0 = QQ * qq
                out_t = out_ts[(p, qq)]
                for kti, soff, qstart, W, av_start in entries:
                    off = qstart - q0
                    last = gi == ng - 1 and (kti, soff) == (entries[-1][0], entries[-1][1])
                    for j in range(2):
                        nc.tensor.matmul(
                            out_t[:, j, off : off + W],
                            vts[p][j][:, kti, :],
                            ex[:, j, soff : soff + W],
                            start=av_start,
                            stop=last,
                            skip_group_check=True,
                        )
                is_last_unit = i >= len(all_groups) - 3  # last unit's diag groups
                if is_last_unit:
                    # final quarter drains in three stages so the tail copy +
                    # DMA overlap the remaining diagonal strips (the diag
                    # groups are ordered so columns complete left-to-right)
                    if gi == ng - 3:
                        for j in range(2):
                            drain(p, qq, j, 0, 128, out_t, nc.scalar, nc.sync)
                    elif gi == ng - 2:
                        for j in range(2):
                            drain(p, qq, j, 128, 256, out_t, nc.scalar, nc.sync)
                    else:
                        for j in range(2):
                            drain(p, qq, j, 256, QQ, out_t, nc.scalar, nc.sync)
                elif gi == ng - 1:
                    for j in range(2):
                        nc.vector.tensor_copy(
                            osbs[(p, j)][:, q0 : q0 + QQ], out_t[:, j, :]
                        )
                        nc.sync.dma_start(
                            o[2 * p + j][:, q0 : q0 + QQ],
                            osbs[(p, j)][:, q0 : q0 + QQ],
                        )

            for i in range(min(LOOKAHEAD, len(all_groups))):
                hp, _, _, _, hentries, _ = all_groups[i]
                hoisted[i] = emit_qk(hp, hentries)
            for rec_i, (p, qq, gi, ng, entries, is_diag) in enumerate(all_groups):
                q0 = QQ * qq
                use_dve = False
                if not is_diag and DVE_NUM > 0:
                    use_dve = nd_counter % DVE_DEN < DVE_NUM
                    nd_counter += 1
                sc = hoisted.pop(rec_i)
                wmax = max(soff + W for _, soff, _, W, _ in entries)
                ex = expp.tile([128, 2, QQ], mybir.dt.bfloat16, tag="ex")
                if use_dve:
                    nc.vector.tensor_scalar(
                        ex[:, :, :wmax].bitcast(mybir.dt.int16),
                        sc[:, :, :wmax],
                        EXP2_A,
                        EXP2_B,
                        mybir.AluOpType.mult,
                        mybir.AluOpType.add,
                    )
                else:
                    nc.scalar.activation(
                        ex[:, :, :wmax],
                        sc[:, :, :wmax],
                        mybir.ActivationFunctionType.Exp,
                        scale=0.125,
                    )
                if rec_i + LOOKAHEAD < len(all_groups):
                    hp, _, _, _, hentries, _ = all_groups[rec_i + LOOKAHEAD]
                    hoisted[rec_i + LOOKAHEAD] = emit_qk(hp, hentries)
                for kti, soff, qstart, W, _ in entries:
                    if qstart == 128 * kti:
                        # diagonal block of both heads: zero out k > q
                        mask_eng.tensor_mul(
                            ex[:, :, soff : soff + 128],
                            ex[:, :, soff : soff + 128],
                            mkt[:, None, :].to_broadcast((128, 2, 128)),
                        )
                if gi == 0:
                    out_ts[(p, qq)] = outp.tile(
                        [D + 1, 2, QQ],
                        mybir.dt.float32,
                        tag="out",
                        name=f"out{p}_{qq}",
                    )
                pending[rec_i] = (p, qq, gi, ng, entries, ex)
                if rec_i - AV_LAG >= 0:
                    emit_av(rec_i - AV_LAG)
            for i in range(max(0, len(all_groups) - AV_LAG), len(all_groups)):
                emit_av(i)

    nc.compile()
    return nc


def _get_program():
    global _prog
    if _prog is None:
        _prog = _build_program()
    return _prog


def _prep_in_maps(q, k, v):
    """Build the 8 per-core input maps from full f32 q, k, v."""
    qf = np.ascontiguousarray(q.reshape(B * H, S, D))
    kf = np.ascontiguousarray(k.reshape(B * H, S, D))
    vf = np.ascontiguousarray(v.reshape(B * H, S, D))
    mask = np.triu(np.ones((128, 128), np.float32)).astype(BF16)
    in_maps = []
    for i in range(NCORES):
        sl = slice(HPC * i, HPC * (i + 1))
        # [HPC, D, S] transposed heads, packed pairwise onto 128 partitions
        qT = qf[sl].transpose(0, 2, 1).astype(BF16).reshape(NPAIR, 128, S)
        kT = kf[sl].transpose(0, 2, 1).astype(BF16).reshape(NPAIR, 128, S)
        vpp = np.ones((HPC, 128, NKT, D + 1), dtype=BF16)
        vpp[:, :, :, :D] = (
            vf[sl].reshape(HPC, NKT, 128, D).transpose(0, 2, 1, 3).astype(BF16)
        )
        in_maps.append({"qT": qT, "kT": kT, "vp": vpp, "mk": mask})
    return in_maps


def _postprocess(results):
    """results: list of 8 dicts with 'o' [HPC, D+1, S] f32 -> full output."""
    o = np.stack([r["o"] for r in results])  # [8, HPC, 65, S]
    o = o.reshape(B * H, D + 1, S).astype(np.float32)
    num = o[:, :D, :]  # [BH, D, S]
    den = o[:, D : D + 1, :]  # [BH, 1, S]
    out = (num / den).transpose(0, 2, 1)  # [BH, S, D]
    return np.ascontiguousarray(out.reshape(B, H, S, D).astype(np.float32))


def run(q, k, v, trace=False, **kwargs):
    from concourse.bass_utils import run_bass_kernel_spmd

    nc = _get_program()
    in_maps = _prep_in_maps(q, k, v)
    res = run_bass_kernel_spmd(
        nc, in_maps, core_ids=list(range(NCORES)), trace=trace, **kwargs
    )
    return _postprocess(res.results), res


def kernel(q, k, v):
    out, _ = run(np.asarray(q), np.asarray(k), np.asarray(v))
    return out


# revision 27
# speedup vs baseline: 1.0413x; 1.0413x over previous
"""Causal attention kernel for TRN2, sharded over batch*heads on 8 NeuronCores.

Problem: B=2, H=16, S=2048, D=64, f32 causal scaled-dot-product attention.

Strategy (per core: 4 heads = 2 head-pairs):
  - Host pre-transposes Q, K to [D, S] (d on partitions), packs two heads
    per 128-partition tile (head A on partitions 0:64, head B on 64:128),
    casts to bf16 (PE runs bf16 at 1 cyc/row vs 4 for f32).
  - QK^T for the two heads runs as two concurrent row-tiled matmuls
    (tile_position auto-derived from base_partition 0 / 64).
  - Host appends a ones-column to V so the softmax denominator falls out of
    the same PE matmul that computes exp(S)@V (M = 65 stationary columns).
  - Work unit: (pair, q-quarter qq of 512, k-tile kt<=4qq+3) strip of
    scoresT [128 k, 2 heads, W<=512 q] in PSUM; one exp covers both
    heads via a [128, 2, W] access pattern straight out of PSUM (scale=1/8
    folded in); no max-subtraction (scores ~ N(0,1), exp cannot overflow);
    diagonal 128x128 blocks masked by one bf16 triu multiply on VectorE for
    both heads.
  - exp is split between ScalarE (true exp ACTIVATE) and VectorE (Schraudolph
    bf16-bit-pattern exp2 via one tensor_scalar mult+add, ~3% elementwise
    error that softmax renormalization mostly cancels): ScalarE alone is the
    bottleneck at ~75us busy while VectorE idles, so every DVE_STRIDE-th
    off-diagonal strip moves to VectorE.
  - PSUM (8 banks): scores triple-buffered (3 x 2 banks) so QK always runs
    two groups ahead of exp; one single-buffered [65, 2, 512] out accumulator
    (2 banks).
  - Startup: PE comes out of reset HAM-throttled (K=4/8, half rate) and only
    unthrottles after sustained dense activity; a zero warmup matmul block
    starts PE activity at ~7us (vs ~10) to pull the unthrottle point in.
    First-strip DMAs (k-tile 0, q quarter 0, mask) are issued first and small.
  - Tail: the final quarter drains cols 0:128 right after their last
    contributing k-tile instead of waiting for the full quarter.
  - Device ships unnormalized [65, S] per head (rows 0-63 numerator^T,
    row 64 denominator); host divides and transposes back.
"""

import numpy as np
import ml_dtypes

B, H, S, D = 2, 16, 2048, 64
NCORES = 8
HPC = (B * H) // NCORES  # heads per core = 4
NPAIR = HPC // 2  # head pairs per core = 2
NKT = S // 128  # 16 k-tiles per head
QQ = 512  # q quarter width (one PSUM bank per head)
DVE_NUM, DVE_DEN = 1, 2  # offload DVE_NUM of every DVE_DEN off-diag strips to VectorE
GPSIMD_MASK = False  # run the diagonal mask multiplies on the idle GPSIMD
NWARM = 6  # zero matmuls at program start to release the PE HAM throttle
NQQ = S // QQ
BF16 = ml_dtypes.bfloat16

_prog = None


def _build_program():
    import concourse.tile as tile
    from concourse import bacc, mybir

    nc = bacc.Bacc(
        "TRN2",
        target_bir_lowering=False,
        debug=False,
        enable_asserts=False,
        num_devices=NCORES,
    )
    # paired layouts: [pair, 128, S] with head 2p on partitions 0:64, head
    # 2p+1 on partitions 64:128
    qT = nc.dram_tensor("qT", [NPAIR, 128, S], mybir.dt.bfloat16, kind="ExternalInput").ap()
    kT = nc.dram_tensor("kT", [NPAIR, 128, S], mybir.dt.bfloat16, kind="ExternalInput").ap()
    vp = nc.dram_tensor("vp", [HPC, 128, NKT, D + 1], mybir.dt.bfloat16, kind="ExternalInput").ap()
    mk = nc.dram_tensor("mk", [128, 128], mybir.dt.bfloat16, kind="ExternalInput").ap()
    o = nc.dram_tensor("o", [HPC, D + 1, S], mybir.dt.float32, kind="ExternalOutput").ap()

    with tile.TileContext(nc) as tc:
        with (
            tc.tile_pool(name="inputs", bufs=1) as inputs,
            tc.tile_pool(name="expp", bufs=12) as expp,
            tc.tile_pool(name="scp", bufs=3, space="PSUM") as scp,
            tc.tile_pool(name="outp", bufs=1, space="PSUM") as outp,
            tc.tile_pool(name="outsb", bufs=6) as outsb,
        ):
            mkt = inputs.tile([128, 128], mybir.dt.bfloat16, tag="mask")
            qts, kts_, vts = [], [], []
            for p in range(NPAIR):
                qt = inputs.tile([128, S], mybir.dt.bfloat16, tag=f"q{p}")
                kt = inputs.tile([128, S], mybir.dt.bfloat16, tag=f"k{p}")
                va = inputs.tile([128, NKT, D + 1], mybir.dt.bfloat16, tag=f"va{p}")
                vb = inputs.tile([128, NKT, D + 1], mybir.dt.bfloat16, tag=f"vb{p}")
                qts.append(qt)
                kts_.append(kt)
                vts.append((va, vb))
            # The first strip (pair 0, quarter 0, k-tile 0) needs k0's first
            # k-tile, q0's first quarter, the mask and V k-tile 0. Issue those
            # first, split across the two HWDGE queues (sync + scalar);
            # everything else follows behind in the same FIFOs so it cannot
            # steal bandwidth from the critical-path transfers.
            nc.sync.dma_start(kts_[0][:, 0:128], kT[0][:, 0:128])
            nc.scalar.dma_start(qts[0][:, 0:256], qT[0][:, 0:256])
            nc.sync.dma_start(mkt[:], mk)
            nc.scalar.dma_start(qts[0][:, 256:QQ], qT[0][:, 256:QQ])
            nc.scalar.dma_start(vts[0][1][:, 0:4], vp[1][:, 0:4])
            nc.sync.dma_start(vts[0][0][:, 0:4], vp[0][:, 0:4])
            nc.sync.dma_start(kts_[0][:, 128:QQ], kT[0][:, 128:QQ])
            nc.sync.dma_start(kts_[0][:, QQ : 2 * QQ], kT[0][:, QQ : 2 * QQ])
            nc.sync.dma_start(qts[0][:, QQ : 2 * QQ], qT[0][:, QQ : 2 * QQ])
            nc.sync.dma_start(vts[0][0][:, 4:8], vp[0][:, 4:8])
            nc.sync.dma_start(vts[0][1][:, 4:8], vp[1][:, 4:8])
            nc.sync.dma_start(kts_[1][:, 0:QQ], kT[1][:, 0:QQ])
            nc.sync.dma_start(qts[1][:, 0:QQ], qT[1][:, 0:QQ])
            nc.sync.dma_start(vts[1][0][:, 0:4], vp[2][:, 0:4])
            nc.sync.dma_start(vts[1][1][:, 0:4], vp[3][:, 0:4])
            nc.sync.dma_start(kts_[0][:, 2 * QQ : S], kT[0][:, 2 * QQ : S])
            nc.sync.dma_start(qts[0][:, 2 * QQ : S], qT[0][:, 2 * QQ : S])
            nc.sync.dma_start(vts[0][0][:, 8:NKT], vp[0][:, 8:NKT])
            nc.sync.dma_start(vts[0][1][:, 8:NKT], vp[1][:, 8:NKT])
            nc.sync.dma_start(kts_[1][:, QQ:S], kT[1][:, QQ:S])
            nc.sync.dma_start(qts[1][:, QQ:S], qT[1][:, QQ:S])
            nc.sync.dma_start(vts[1][0][:, 4:NKT], vp[2][:, 4:NKT])
            nc.sync.dma_start(vts[1][1][:, 4:NKT], vp[3][:, 4:NKT])

            # PE warmup: the HAM throttle boots at K=4/8 (half rate) and only
            # releases after sustained activity. Burn NWARM zero matmuls into
            # the (not yet used) out accumulator buffer so PE activity starts
            # as soon as the engines come up, well before the first real QK's
            # input DMA lands.
            if NWARM:
                # W=128 zero matmuls into the sc pool's first slot: they run
                # 7.0-8.3us while the first input DMA is still in flight, so
                # the HAM activity clock starts ~1.3us earlier at zero cost
                # (slot 0's next real user only runs at ~12us).
                warm_sb = inputs.tile([128, 128], mybir.dt.bfloat16, tag="warm")
                nc.gpsimd.memset(warm_sb[:], 0.0)
                warm_ps = scp.tile(
                    [128, 2, QQ], mybir.dt.float32, tag="sc", name="warm_ps"
                )
                for _ in range(NWARM):
                    nc.tensor.matmul(
                        warm_ps[:, 0, 0:128],
                        warm_sb[:],
                        warm_sb[:],
                        start=True,
                        stop=True,
                        skip_group_check=True,
                    )

            osbs = {}
            for p in range(NPAIR):
                for jj in range(2):
                    osbs[(p, jj)] = outsb.tile(
                        [D + 1, S], mybir.dt.float32, tag="osb", name=f"osb{p}_{jj}"
                    )
            # interleave the two pairs' quarters so both engines always have
            # independent work to fill dependency gaps
            order = [(0, 0), (0, 1), (1, 0), (0, 2), (1, 1), (0, 3), (1, 2), (1, 3)]
            # flatten (pair, quarter) into a list of strip groups; each group
            # is one score tile + one exp call covering one or two k-tiles
            # (the W=384 and W=128 diagonal strips share a tile). Entries are
            # (kti, soff, qstart, W); av_start flags are derived below from
            # 128-col coverage so the first AV touching each column range
            # resets PSUM.
            all_groups = []
            for p, qq in order:
                q0 = QQ * qq

                def ent(kti, soff, qstart=None, W=None, q0=q0):
                    qs = max(q0, 128 * kti) if qstart is None else qstart
                    w = q0 + QQ - qs if W is None else W
                    return (kti, soff, qs, w)

                if (p, qq) == (0, 0):
                    # split the first diagonal strip in half so the very first
                    # QK only needs q[0:256] + k-tile 0 from DRAM: the first
                    # exp fires ~1us earlier
                    groups = [
                        [ent(0, 0, 0, 256)],
                        [ent(0, 0, 256, 256)],
                        [ent(2, 0)],
                        [ent(1, 0), ent(3, 384)],
                    ]
                elif (p, qq) == order[-1]:
                    # last unit: order the diagonal strips so output columns
                    # complete left-to-right and can drain in three stages
                    groups = [[ent(kti, 0)] for kti in range(4 * qq)]
                    groups.append([ent(4 * qq, 0)])
                    groups.append([ent(4 * qq + 1, 0), ent(4 * qq + 3, 384)])
                    groups.append([ent(4 * qq + 2, 0)])
                else:
                    groups = [[ent(kti, 0)] for kti in range(4 * qq)]
                    groups.append([ent(4 * qq, 0)])  # W=512 diagonal
                    groups.append([ent(4 * qq + 2, 0)])  # W=256 diagonal
                    groups.append([ent(4 * qq + 1, 0), ent(4 * qq + 3, 384)])
                for gi, group in enumerate(groups):
                    # start=True lazily marks the whole 2KB PSUM bank as
                    # pending-zero, so only the quarter's very first AV needs
                    # it; later writes to untouched bytes still zero-fill.
                    entries = [
                        (kti, soff, qs, w, gi == 0 and ei == 0)
                        for ei, (kti, soff, qs, w) in enumerate(group)
                    ]
                    is_diag = all(128 * kti >= q0 for kti, _, _, _, _ in entries)
                    all_groups.append((p, qq, gi, len(groups), entries, is_diag))

            # Fast-exp2 constants (Schraudolph): int16(x*A + B) reinterpreted
            # as bf16 is 2^(x*log2e/8) ~ exp(x/8)*(1 +- 3%); the softmax
            # renormalization cancels most of the approximation error.
            EXP2_A = 128.0 / float(np.log(2.0)) / 8.0
            EXP2_B = 16256.0 - 366393.0 / 65536.0

            def emit_qk(p, entries):
                qt, kt = qts[p], kts_[p]
                sc = scp.tile(
                    [128, 2, QQ], mybir.dt.float32, tag="sc", name="sc_tile"
                )
                for kti, soff, qstart, W, _ in entries:
                    for j in range(2):
                        pb = 64 * j
                        nc.tensor.matmul(
                            sc[:, j, soff : soff + W],
                            kt[pb : pb + 64, 128 * kti : 128 * kti + 128],
                            qt[pb : pb + 64, qstart : qstart + W],
                            start=True,
                            stop=True,
                        )
                return sc

            def drain(p, qq, j, c0, c1, out_t, eng, dmaq):
                """Copy out cols [c0:c1) of head j to SBUF and DMA them out."""
                q0 = QQ * qq
                eng.copy(
                    osbs[(p, j)][:, q0 + c0 : q0 + c1], out_t[:, j, c0:c1]
                )
                dmaq.dma_start(
                    o[2 * p + j][:, q0 + c0 : q0 + c1],
                    osbs[(p, j)][:, q0 + c0 : q0 + c1],
                )

            nd_counter = 0
            out_ts = {}
            mask_eng = nc.gpsimd if GPSIMD_MASK else nc.vector
            # Software-pipeline: QK runs two groups ahead of exp and AV runs
            # one group behind it. The scheduler is a priority heap keyed by
            # emission order, so emitting QK(i+2) before AV(i-1) lets PE
            # start the next unit's scores while the previous unit's AV
            # backlog drains — otherwise ScalarE starves ~2.5us at every
            # (pair, quarter) boundary. Lagging AV by one group means the AV
            # almost never waits on its exp, so the PE pipeline doesn't drain
            # (each exposed drain costs the 173ns SBUF access latency).
            LOOKAHEAD = 2
            AV_LAG = 1
            hoisted = {}
            pending = {}

            def emit_av(i):
                p, qq, gi, ng, entries, ex = pending.pop(i)
                q0 = QQ * qq
                out_t = out_ts[(p, qq)]
                for kti, soff, qstart, W, av_start in entries:
                    off = qstart - q0
                    last = gi == ng - 1 and (kti, soff) == (entries[-1][0], entries[-1][1])
                    for j in range(2):
                        nc.tensor.matmul(
                            out_t[:, j, off : off + W],
                            vts[p][j][:, kti, :],
                            ex[:, j, soff : soff + W],
                            start=av_start,
                            stop=last,
                            skip_group_check=True,
                        )
                is_last_unit = i >= len(all_groups) - 3  # last unit's diag groups
                if is_last_unit:
                    # final quarter drains in three stages so the tail copy +
                    # DMA overlap the remaining diagonal strips (the diag
                    # groups are ordered so columns complete left-to-right)
                    if gi == ng - 3:
                        for j in range(2):
                            drain(p, qq, j, 0, 128, out_t, nc.scalar, nc.sync)
                    elif gi == ng - 2:
                        for j in range(2):
                            drain(p, qq, j, 128, 256, out_t, nc.scalar, nc.sync)
                    else:
                        for j in range(2):
                            drain(p, qq, j, 256, QQ, out_t, nc.scalar, nc.sync)
                elif gi == ng - 1:
                    for j in range(2):
                        nc.vector.tensor_copy(
                            osbs[(p, j)][:, q0 : q0 + QQ], out_t[:, j, :]
                        )
                        nc.sync.dma_start(
                            o[2 * p + j][:, q0 : q0 + QQ],
                            osbs[(p, j)][:, q0 : q0 + QQ],
                        )

            for i in range(min(LOOKAHEAD, len(all_groups))):
                hp, _, _, _, hentries, _ = all_groups[i]
                hoisted[i] = emit_qk(hp, hentries)
            for rec_i, (p, qq, gi, ng, entries, is_diag) in enumerate(all_groups):
                q0 = QQ * qq
                use_dve = False
                if not is_diag and DVE_NUM > 0:
                    use_dve = nd_counter % DVE_DEN >= DVE_DEN - DVE_NUM
                    nd_counter += 1
                sc = hoisted.pop(rec_i)
                wmax = max(soff + W for _, soff, _, W, _ in entries)
                ex = expp.tile([128, 2, QQ], mybir.dt.bfloat16, tag="ex")
                if use_dve:
                    nc.vector.tensor_scalar(
                        ex[:, :, :wmax].bitcast(mybir.dt.int16),
                        sc[:, :, :wmax],
                        EXP2_A,
                        EXP2_B,
                        mybir.AluOpType.mult,
                        mybir.AluOpType.add,
                    )
                else:
                    nc.scalar.activation(
                        ex[:, :, :wmax],
                        sc[:, :, :wmax],
                        mybir.ActivationFunctionType.Exp,
                        scale=0.125,
                    )
                if rec_i + LOOKAHEAD < len(all_groups):
                    hp, _, _, _, hentries, _ = all_groups[rec_i + LOOKAHEAD]
                    hoisted[rec_i + LOOKAHEAD] = emit_qk(hp, hentries)
                for kti, soff, qstart, W, _ in entries:
                    if qstart == 128 * kti:
                        # diagonal block of both heads: zero out k > q
                        mask_eng.tensor_mul(
                            ex[:, :, soff : soff + 128],
                            ex[:, :, soff : soff + 128],
                            mkt[:, None, :].to_broadcast((128, 2, 128)),
                        )
                if gi == 0:
                    out_ts[(p, qq)] = outp.tile(
                        [D + 1, 2, QQ],
                        mybir.dt.float32,
                        tag="out",
                        name=f"out{p}_{qq}",
                    )
                pending[rec_i] = (p, qq, gi, ng, entries, ex)
                if rec_i - AV_LAG >= 0:
                    emit_av(rec_i - AV_LAG)
            for i in range(max(0, len(all_groups) - AV_LAG), len(all_groups)):
                emit_av(i)

    nc.compile()
    return nc


def _get_program():
    global _prog
    if _prog is None:
        _prog = _build_program()
    return _prog


def _prep_in_maps(q, k, v):
    """Build the 8 per-core input maps from full f32 q, k, v."""
    qf = np.ascontiguousarray(q.reshape(B * H, S, D))
    kf = np.ascontiguousarray(k.reshape(B * H, S, D))
    vf = np.ascontiguousarray(v.reshape(B * H, S, D))
    mask = np.triu(np.ones((128, 128), np.float32)).astype(BF16)
    in_maps = []
    for i in range(NCORES):
        sl = slice(HPC * i, HPC * (i + 1))
        # [HPC, D, S] transposed heads, packed pairwise onto 128 partitions
        qT = qf[sl].transpose(0, 2, 1).astype(BF16).reshape(NPAIR, 128, S)
        kT = kf[sl].transpose(0, 2, 1).astype(BF16).reshape(NPAIR, 128, S)
        vpp = np.ones((HPC, 128, NKT, D + 1), dtype=BF16)
        vpp[:, :, :, :D] = (
            vf[sl].reshape(HPC, NKT, 128, D).transpose(0, 2, 1, 3).astype(BF16)
        )
        in_maps.append({"qT": qT, "kT": kT, "vp": vpp, "mk": mask})
    return in_maps


def _postprocess(results):
    """results: list of 8 dicts with 'o' [HPC, D+1, S] f32 -> full output."""
    o = np.stack([r["o"] for r in results])  # [8, HPC, 65, S]
    o = o.reshape(B * H, D + 1, S).astype(np.float32)
    num = o[:, :D, :]  # [BH, D, S]
    den = o[:, D : D + 1, :]  # [BH, 1, S]
    out = (num / den).transpose(0, 2, 1)  # [BH, S, D]
    return np.ascontiguousarray(out.reshape(B, H, S, D).astype(np.float32))


def run(q, k, v, trace=False, **kwargs):
    from concourse.bass_utils import run_bass_kernel_spmd

    nc = _get_program()
    in_maps = _prep_in_maps(q, k, v)
    res = run_bass_kernel_spmd(
        nc, in_maps, core_ids=list(range(NCORES)), trace=trace, **kwargs
    )
    return _postprocess(res.results), res


def kernel(q, k, v):
    out, _ = run(np.asarray(q), np.asarray(k), np.asarray(v))
    return out
